# revision 69
# baseline (speedup 1.0000x reference)
"""AttnBlock (GroupNorm + single-head self-attention + residual) on 8 TRN2 cores.

Sharding: data-parallel over (batch b, query-half h) -> 8 shards. Each core
receives the full [C, N] image of its batch (columns rolled so that its own
query half always occupies columns 0:NQ), computes GroupNorm stats + K/V over
the whole image, Q over its half, and a flash-style attention in which scores
are produced directly transposed (S^T = K^T.T @ Q^T tiles) so no PE transposes
of P are needed. Attention matmuls (scores, P.V, and the softmax denominator
via a ones-vector) run in fp8e4 DoubleRow mode (256-deep contraction per
instruction); projections run in bf16 with fp32 PSUM accumulation. Softmax is
computed as exp(s*scale - SHIFT) -- the constant shift cancels between
numerator and denominator and keeps exp outputs in fp8e4 range.
"""

import os
import sys

import numpy as np

for _p in ("/opt/trn_rl_repo", "/root/.axon_site/_ro/trn_rl_repo"):
    if os.path.isdir(_p) and _p not in sys.path:
        sys.path.insert(0, _p)

import concourse.bass as bass  # noqa: E402
import concourse.tile as tile  # noqa: E402
from concourse import bacc, mybir  # noqa: E402
from concourse.masks import make_identity  # noqa: E402

# The agent image's antenv lacks axon_hooks; if BASS_TRACE is set in the
# environment, run_bass_kernel_spmd would crash importing it. Provide a stub
# (profiling degrades gracefully to "hook isn't registered").
try:
    import antenv.axon_hooks  # noqa: F401
except ImportError:
    import types as _types

    _m = _types.ModuleType("antenv.axon_hooks")
    _h = [None]
    _m.set_axon_ntff_profile_hook = lambda h: _h.__setitem__(0, h)
    _m.get_axon_ntff_profile_hook = lambda: _h[0]
    sys.modules["antenv.axon_hooks"] = _m

B, C, H, W = 4, 256, 64, 64
N = H * W  # 4096 pixels
NQ = N // 2  # 2048 queries per core
G = 32  # groups
CPG = C // G  # 8 channels per group
EPS = 1e-5
NCORES = 8
SCALE = float(C) ** -0.5  # 0.0625
SHIFT = 3.0  # exp(s*SCALE - SHIFT): cancels in softmax, keeps P in fp8 range
# (TRN FP8_EXP4 max normal is +-240: values >= 256 hit inf, unlike OCP E4M3FN)

F32 = mybir.dt.float32
BF16 = mybir.dt.bfloat16
FP8 = mybir.dt.float8e4

QB = 512  # query block (free dim of S^T / PV matmuls)
NQB = NQ // QB  # 4 query blocks
NKT = N // 128  # 32 key tiles
NPAIR = NKT // 2  # 16 key-tile pairs (fp8 DoubleRow granularity)
NNB = N // QB  # 8 pixel blocks for K/V projections
P = 128

Act = mybir.ActivationFunctionType
Alu = mybir.AluOpType
Axis = mybir.AxisListType
DR = mybir.MatmulPerfMode.DoubleRow

_NC = None
LAST_RESULTS = None


def _body(tc, d):
    nc = tc.nc
    x_d = d["x"]
    out_d = d["out"]

    const = tc.alloc_tile_pool(name="const", bufs=1)
    stage = tc.alloc_tile_pool(name="stage", bufs=2)
    small = tc.alloc_tile_pool(name="small", bufs=1)
    pblk = tc.alloc_tile_pool(name="pblk", bufs=2)
    work = tc.alloc_tile_pool(name="work", bufs=2)
    # PSUM budget (8 banks): sc 2x[P,1024]=4, acc 2x[P,512]=2, dps 1, po 1
    ps_sc = tc.alloc_tile_pool(name="ps_sc", bufs=2, space="PSUM")
    ps_acc = tc.alloc_tile_pool(name="ps_acc", bufs=2, space="PSUM")
    ps_d = tc.alloc_tile_pool(name="ps_d", bufs=1, space="PSUM")
    ps_o = tc.alloc_tile_pool(name="ps_o", bufs=1, space="PSUM")

    # ---- x in SBUF first: [128, 2(ch), 4096] bf16 (cast on host), chunked
    # across two DMA queues (SP for ch0, Pool for ch1) so bn_stats overlaps
    # the transfer; the fp8 copy of x (projection operand) follows on SP ----
    x_sb = const.tile([P, 2, N], BF16)
    x_bf = x_sb
    x_src = x_d.ap().rearrange("(h p) n -> p h n", p=P)
    x8_sb = const.tile([P, 2, N], FP8)
    x8_src = d["x8"].ap().rearrange("(h p) n -> p h n", p=P)
    # GroupNorm input statistics, split across engines: DVE bn_stats covers
    # pixel slices 0..5 of each channel half, the ACT accumulator (Identity /
    # Square with accum_out) covers slices 6..7 -- the two run concurrently,
    # nearly halving the serial stats chain.
    # ch0 chunks + x8 ride the SP DMA queue; ch1 chunks ride the Pool queue
    # (cheap dma dispatch) ahead of the weight loads, so both channel halves
    # land concurrently. ACT digests chunk 0 of each half (lands first) via
    # its accumulator; DVE bn_stats digest chunks 1-3.
    bn_st = [small.tile([P, 6, 6], F32, name=f"bnst_{ch}") for ch in range(2)]
    s1 = [small.tile([P, 1], F32, name=f"s1_{ch}") for ch in range(2)]
    s2 = [small.tile([P, 1], F32, name=f"s2_{ch}") for ch in range(2)]
    for c in range(4):
        for ch in range(2):
            sl = (slice(None), ch, slice(c * 2 * QB, (c + 1) * 2 * QB))
            eng = nc.sync if ch == 0 else nc.gpsimd
            eng.dma_start(out=x_sb[sl], in_=x_src[sl])
            if c == 0:
                junk = stage.tile([P, 2 * QB], BF16, name="junk", tag="junk")
                nc.scalar.activation(
                    junk, x_sb[:, ch, 0:2 * QB], Act.Identity,
                    accum_out=s1[ch],
                )
                junk2 = stage.tile([P, 2 * QB], BF16, name="junk", tag="junk")
                nc.scalar.activation(
                    junk2, x_sb[:, ch, 0:2 * QB], Act.Square,
                    accum_out=s2[ch],
                )
            else:
                for j in (2 * c, 2 * c + 1):
                    nc.vector.bn_stats(
                        out=bn_st[ch][:, j - 2, :],
                        in_=x_sb[:, ch, j * QB:(j + 1) * QB],
                    )
    nc.sync.dma_start(out=x8_sb, in_=x8_src)

    # PE warm-up: keep the HAM activity monitor busy during the DMA/stats
    # window so projections and attention run at full clock from the start.
    wu_w = const.tile([P, P], BF16)
    nc.vector.memset(wu_w, 0.0)
    wu_x = const.tile([P, QB], BF16)
    nc.vector.memset(wu_x, 0.0)
    wu_ps = ps_sc.tile([P, 2, QB], F32, name="wu_ps", tag="sc")

    def warm(n):
        for _ in range(n):
            nc.tensor.matmul(
                wu_ps[:, 0, :], lhsT=wu_w, rhs=wu_x, start=True, stop=True
            )

    warm(40)

    # ---- constants ----
    one11 = const.tile([1, 1], F32)
    nc.vector.memset(one11, 1.0)
    # ones pair for the fp8 denominator matmuls: the dual-fp8 ldweights ISA
    # requires the pair as the innermost AP dim with byte-step % 16 == 0,
    # hence the padded [P, 2, 16] layout viewed as [P, 1, 2]
    ones8_t = const.tile([P, 2, 16], FP8)
    nc.vector.memset(ones8_t, 1.0)
    ones8 = ones8_t[:, :, 0:16]  # 16 identical denominator rows in psum
    ones_row = const.tile([1, P], F32)
    nc.vector.memset(ones_row, 1.0)
    eps_col = const.tile([P, 1], F32)
    nc.vector.memset(eps_col, EPS)
    shift_col = const.tile([P, 1], F32)
    nc.vector.memset(shift_col, -SHIFT)
    # preload the ACT exp and sqrt tables before the dance/attention need them
    warm11 = small.tile([1, 1], F32)
    nc.scalar.activation(warm11, one11, Act.Exp, scale=1.0)
    warm12 = small.tile([1, 1], F32)
    nc.scalar.activation(warm12, one11, Act.Sqrt, scale=1.0)

    # ---- weights first on the Pool DMA queue (their bf16 casts gate the
    # whole dance), then the small norm/bias vectors ----
    w_bf = {}
    for nm in ("wqt", "wkt", "wvt", "wot"):
        wstg = stage.tile([P, 2, C], F32, name="wstg", tag="wstg")
        nc.gpsimd.dma_start(
            out=wstg, in_=d[nm].ap().rearrange("(h p) co -> p h co", p=P)
        )
        wb = const.tile([P, 2, C], BF16, name=f"{nm}_bf")
        for ch in range(2):
            # q/k casts on ACT (they gate the w8 fp8 weights -> projections);
            # v/o casts on the otherwise-idle Pool engine
            if nm in ("wqt", "wkt"):
                nc.scalar.copy(wb[:, ch, :], wstg[:, ch, :])
            else:
                nc.gpsimd.tensor_copy(out=wb[:, ch, :], in_=wstg[:, ch, :])
        w_bf[nm] = wb

    # group indicator matrices + gamma/beta columns (host-prepared constants)
    gind_sb = const.tile([P, 2 * G], F32)
    nc.gpsimd.dma_start(out=gind_sb, in_=d["gind"].ap())
    gindT_sb = const.tile([G, 2 * P], F32)
    nc.gpsimd.dma_start(out=gindT_sb, in_=d["gindT"].ap())
    gb_sb = const.tile([P, 4], F32)
    nc.gpsimd.dma_start(out=gb_sb, in_=d["gb_cols"].ap())

    # per-partition bias columns [128,1] x 2 channel-halves (bk cancels in
    # softmax -- a per-channel K offset shifts every score of a query equally)
    bias_cols = {}
    for nm in ("bq", "bv", "bo"):
        cols = []
        for ch in range(2):
            t = const.tile([P, 1], F32, name=f"{nm}_{ch}")
            nc.gpsimd.dma_start(out=t, in_=d[nm][ch * P:(ch + 1) * P, :])
            cols.append(t)
        bias_cols[nm] = cols

    # ---- GroupNorm statistics: combine the DVE bn_stats aggregates (6
    # slices = 3072 px) with the ACT accumulator sums (2 slices = 1024 px)
    # into raw per-channel sums SX = sum(x), SXX = sum(x^2) ----
    NPX6 = float(6 * QB)
    sums4 = small.tile([P, 4], F32, name="sums4")  # [sx0, sxx0, sx1, sxx1]
    for ch in range(2):
        m = small.tile([P, 2], F32, name=f"mv_{ch}")
        nc.vector.bn_aggr(out=m, in_=bn_st[ch])
        nc.vector.scalar_tensor_tensor(
            out=sums4[:, 2 * ch:2 * ch + 1], in0=m[:, 0:1], scalar=NPX6,
            in1=s1[ch], op0=Alu.mult, op1=Alu.add,
        )
        msq = small.tile([P, 1], F32, name=f"msq_{ch}")
        nc.vector.tensor_mul(msq, m[:, 0:1], m[:, 0:1])
        vpm = small.tile([P, 1], F32, name=f"vpm_{ch}")
        nc.vector.tensor_add(vpm, m[:, 1:2], msq)
        nc.vector.scalar_tensor_tensor(
            out=sums4[:, 2 * ch + 1:2 * ch + 2], in0=vpm, scalar=NPX6,
            in1=s2[ch], op0=Alu.mult, op1=Alu.add,
        )

    warm(25)  # keep the PE activity monitor warm through the stats dance

    # group sums via indicator matmul: [32, 2] = sum over the 8 channels of
    # each group of [SX, SXX] -- no row transposes needed, everything stays
    # in column space
    MPG = float(CPG * N)
    gps = ps_o.tile([G, 2], F32, name="gps", tag="po")
    for h in range(2):
        nc.tensor.matmul(
            gps, lhsT=gind_sb[:, h * G:(h + 1) * G],
            rhs=sums4[:, 2 * h:2 * h + 2], start=(h == 0), stop=(h == 1),
        )
    warm(8)  # cover the group-stat DVE chain latency
    rm = small.tile([G, 2], F32, name="rm")  # [rstd_g | mean_g]
    nc.vector.tensor_scalar_mul(rm[:, 1:2], gps[:, 0:1], 1.0 / MPG)
    msq_g = small.tile([G, 1], F32, name="msq_g")
    nc.vector.tensor_mul(msq_g, rm[:, 1:2], rm[:, 1:2])
    var_g = small.tile([G, 1], F32, name="var_g")
    nc.vector.scalar_tensor_tensor(
        out=var_g, in0=gps[:, 1:2], scalar=1.0 / MPG, in1=msq_g, op0=Alu.mult,
        op1=Alu.subtract,
    )
    sq_g = small.tile([G, 1], F32, name="sq_g")
    nc.scalar.activation(sq_g, var_g, Act.Sqrt, bias=eps_col[0:G, :], scale=1.0)
    nc.vector.reciprocal(rm[:, 0:1], sq_g)

    # broadcast group -> channel columns via the transposed indicator:
    # ecols[:, 2h:2h+2] = [rstd_c | mean_c] for input-channel half h
    eps_ps = ps_o.tile([P, 4], F32, name="ecols_ps", tag="po")
    for h in range(2):
        nc.tensor.matmul(
            eps_ps[:, 2 * h:2 * h + 2], lhsT=gindT_sb[:, h * P:(h + 1) * P],
            rhs=rm, start=True, stop=True,
        )
    ecols = small.tile([P, 4], F32, name="ecols")
    nc.vector.tensor_copy(out=ecols, in_=eps_ps)

    # a = gamma * rstd ; b = beta - mean * a   (all [128,1] columns)
    ab_cols = {"a": [], "b": []}
    b_bf = []
    for h in range(2):
        a_col = small.tile([P, 1], F32, name=f"a_col_{h}")
        nc.vector.tensor_mul(a_col, ecols[:, 2 * h:2 * h + 1], gb_sb[:, h:h + 1])
        ma = small.tile([P, 1], F32, name=f"ma_{h}")
        nc.vector.tensor_mul(ma, ecols[:, 2 * h + 1:2 * h + 2], a_col)
        b_col = small.tile([P, 1], F32, name=f"b_col_{h}")
        nc.vector.tensor_sub(b_col, gb_sb[:, 2 + h:3 + h], ma)
        ab_cols["a"].append(a_col)
        ab_cols["b"].append(b_col)
        t = small.tile([P, 1], BF16, name=f"b_bf_{h}")
        nc.vector.tensor_copy(out=t, in_=b_col)
        b_bf.append(t)
    warm(8)  # keep the PE clock up while the DVE finishes w8 / b_bf

    def matvec_bias(wname, rhs_cols, bias_add, out_dt, out_name):
        outs = []
        for co in range(2):
            pe = ps_o.tile([P, 1], F32, name="pe_mv", tag="po")
            for ci in range(2):
                nc.tensor.matmul(
                    pe, lhsT=w_bf[wname][:, ci, co * P:(co + 1) * P],
                    rhs=rhs_cols[ci], start=(ci == 0), stop=(ci == 1),
                )
            t = small.tile([P, 1], out_dt, name=f"{out_name}_{co}")
            nc.scalar.activation(
                t, pe, Act.Identity, bias=bias_add[co], scale=1.0
            )
            outs.append(t)
        return outs

    # fp8 projection weights: w8 = w_bf * (a * 8) per input channel (the *8
    # centers the ~N(0, 1/256) folded weights in fp8e4's normal range; the
    # projection epilogues undo it with scale=1/8). Single-scalar muls: the
    # dual-scalar tensor_scalar form lowers to a 4x-slower DVE path.
    a8_cols = []
    for ci in range(2):
        t = small.tile([P, 1], F32, name=f"a8_{ci}")
        nc.vector.tensor_scalar_mul(t, ab_cols["a"][ci], 8.0)
        a8_cols.append(t)
    w8 = {}
    for wname in ("wqt", "wkt", "wvt"):
        w = const.tile([P, 2, C], FP8, name=f"{wname}_8")
        for ci in range(2):
            nc.vector.tensor_scalar_mul(
                w[:, ci, :], w_bf[wname][:, ci, :], a8_cols[ci]
            )
        w8[wname] = w

    be_q = matvec_bias("wqt", b_bf, bias_cols["bq"], F32, "be_q")
    # epilogue constants (needed only at the first qb epilogue): issued here
    # so their PE/ACT ping-pong hides under the projection matmuls instead of
    # blocking the attention start
    vbv_bf = matvec_bias("wvt", b_bf, bias_cols["bv"], BF16, "vbv")
    bo_eff = matvec_bias("wot", vbv_bf, bias_cols["bo"], F32, "bo_eff")

    # ---- projections (fp8 DoubleRow matmuls: 256-deep contraction/instr) ----
    # K^T [C, N] / Q^T [C, NQ] fp8: psum[co,nb] = w8[:, :, co].T @ x8[:, :, nb]
    # V [N, C] fp8 (bias folded into bo_eff): psum[nt] = x8_chunk.T @ w8v.
    # Q/K and V are interleaved per pixel block on two separate psum rings
    # (sc for Q/K pairs, acc for V) so neither cast engine stalls the PE;
    # casts alternate ACT / DVE.
    k_sb = const.tile([P, 2, N], FP8)
    q_sb = const.tile([P, 2, NQ], FP8)
    v_sb = const.tile([P, NKT, C], FP8)
    v_flat = v_sb.rearrange("p k c -> p (k c)")
    # K (all blocks) + the first Q block run before attention; V is woven
    # into qb0, Q blocks 1..3 into qbs 0..2
    for nb in range(NNB):
        if nb < 1:
            pq = ps_sc.tile([P, 2, QB], F32, name="pq", tag="sc")
            for co in range(2):
                nc.tensor.matmul(
                    pq[:, co, :], lhsT=w8["wqt"][:, :, co * P:(co + 1) * P],
                    rhs=x8_sb[:, :, nb * QB:(nb + 1) * QB],
                    start=True, stop=True, perf_mode=DR,
                )
            nc.scalar.activation(
                q_sb[:, 0, nb * QB:(nb + 1) * QB], pq[:, 0, :], Act.Identity,
                bias=be_q[0], scale=0.125,
            )
            nc.vector.tensor_scalar(
                out=q_sb[:, 1, nb * QB:(nb + 1) * QB], in0=pq[:, 1, :],
                scalar1=0.125, scalar2=be_q[1], op0=Alu.mult, op1=Alu.add,
            )
        # K blocks on two independent psum rings (even -> sc ring with a
        # whole-tile ACT cast, odd -> acc ring halves with DVE casts): two
        # producer-consumer lanes stall less than one, keeping the PE dense
        # enough that the activity monitor holds full clock. K needs no bias
        # (a per-channel K offset cancels in softmax).
        if nb % 2 == 0:
            pk = ps_sc.tile([P, 2, QB], F32, name="pk", tag="sc")
            for co in range(2):
                nc.tensor.matmul(
                    pk[:, co, :], lhsT=w8["wkt"][:, :, co * P:(co + 1) * P],
                    rhs=x8_sb[:, :, nb * QB:(nb + 1) * QB],
                    start=True, stop=True, perf_mode=DR,
                )
            nc.scalar.mul(k_sb[:, :, nb * QB:(nb + 1) * QB], pk, 0.125)
        else:
            for co in range(2):
                pko = ps_acc.tile([P, QB], F32, name="pko", tag="acc")
                nc.tensor.matmul(
                    pko, lhsT=w8["wkt"][:, :, co * P:(co + 1) * P],
                    rhs=x8_sb[:, :, nb * QB:(nb + 1) * QB],
                    start=True, stop=True, perf_mode=DR,
                )
                nc.vector.tensor_scalar_mul(
                    k_sb[:, co, nb * QB:(nb + 1) * QB], pko, 0.125
                )
        if nb % 2 == 0:
            # idle matmul: keeps the PE activity monitor from dropping the
            # clock to mid-pstate during the cast-paced projection phase
            wu2 = ps_o.tile([P, QB], F32, name="wu2", tag="po")
            nc.tensor.matmul(wu2, lhsT=wu_w, rhs=wu_x, start=True, stop=True)

    # switch the ACT table to Exp now (after the projection-phase Copy casts),
    # so the load doesn't stall the first attention exp
    warm13 = small.tile([1, 1], F32, name="warm13")
    nc.scalar.activation(warm13, one11, Act.Exp, scale=1.0)

    # ---- attention, per query block; pair-granular software pipeline with a
    # deferred epilogue. The softmax division is commuted through the
    # out-projection: out = (wo @ (P.V)) * (1/denom) + bo_eff + x. The
    # denominator is accumulated on the PE by ones-vector fp8 matmuls, so the
    # DVE does no O(N^2) work at all.
    def epilogue(qb, dps, aps, final=False):
        # casts first: they release the PV accumulator banks. Mid-run both go
        # on DVE (ACT is exp-saturated); for the final epilogue ACT is free,
        # so the casts run in parallel on both engines.
        at_sb = [
            work.tile([P, QB], BF16, name="at_sb", tag="at_sb", bufs=4)
            for _ in range(2)
        ]
        den_r = work.tile([1, QB], F32, name="den_r", tag="den_r")
        den_b = work.tile([P, QB], F32, name="den_b", tag="den_b", bufs=2)
        if final:
            # recip/broadcast first: they only need dps, and the 1us Pool
            # broadcast then overlaps the at_sb casts in the serial tail
            nc.vector.reciprocal_approx_fast(out=den_r, in_=dps[0:1, :])
            nc.gpsimd.partition_broadcast(den_b, den_r)
            nc.vector.tensor_copy(out=at_sb[0], in_=aps[0])
            nc.scalar.copy(at_sb[1], aps[1])
        else:
            # casts first: they free the PV accumulator banks the next qb's
            # PV matmuls are waiting on
            nc.vector.tensor_copy(out=at_sb[0], in_=aps[0])
            nc.vector.tensor_copy(out=at_sb[1], in_=aps[1])
            nc.vector.reciprocal_approx_fast(out=den_r, in_=dps[0:1, :])
            nc.gpsimd.partition_broadcast(den_b, den_r)
        if final:
            # the sc ring is idle now: both out-proj halves go in one tile so
            # the second matmul doesn't wait on the first's consumer
            pof = ps_sc.tile([P, 2, QB], F32, name="pof", tag="sc")
            pos = [pof[:, 0, :], pof[:, 1, :]]
        for co in range(2):
            if final:
                po = pos[co]
            else:
                po = ps_o.tile([P, QB], F32, name="po", tag="po")
            for ci in range(2):
                nc.tensor.matmul(
                    po, lhsT=w_bf["wot"][:, ci, co * P:(co + 1) * P],
                    rhs=at_sb[ci], start=(ci == 0), stop=(ci == 1),
                )
            t1 = work.tile([P, QB], F32, name="t1", tag="t1")
            nc.vector.tensor_mul(t1, po, den_b)
            res = work.tile([P, QB], F32, name="res", tag="res", bufs=4)
            nc.vector.scalar_tensor_tensor(
                out=res, in0=t1, scalar=bo_eff[co],
                in1=x_sb[:, co, qb * QB:(qb + 1) * QB], op0=Alu.add, op1=Alu.add,
            )
            nc.sync.dma_start(
                out=out_d[co * P:(co + 1) * P, qb * QB:(qb + 1) * QB], in_=res
            )

    pending = None
    for qb in range(NQB):
        p_sb = pblk.tile([P, NKT, QB], FP8, name="p_sb")
        dps = ps_d.tile([16, QB], F32, name="dps")
        aps = [
            ps_acc.tile([P, QB], F32, name="aps", tag="acc") for _ in range(2)
        ]
        for j in range(NPAIR + 1):
            if j == 1 and pending is not None:
                # previous qb's epilogue goes FIRST so its at_sb casts
                # precede this qb's PV writes into the recycled psum banks
                epilogue(*pending)
                pending = None
            if qb == 0 and j < NPAIR:
                # V projection woven into qb0's attention: pv(j) only needs
                # v_sb[2j:2j+2] one iteration later, so the PE fills its
                # exp-wait slack with V work instead of idling; psum rides
                # the otherwise-idle ps_o bank, casts ride the idle DVE
                pvw = ps_o.tile([P, QB], F32, name="pvw", tag="po")
                for n2 in range(2):
                    nt = 2 * j + n2
                    nc.tensor.matmul(
                        pvw[:, n2 * C:(n2 + 1) * C],
                        lhsT=x8_sb[:, :, nt * P:(nt + 1) * P],
                        rhs=w8["wvt"], start=True, stop=True, perf_mode=DR,
                    )
                nc.vector.tensor_scalar_mul(
                    v_flat[:, 2 * j * C:(2 * j + 2) * C], pvw, 0.125
                )
            if qb < NQB - 1 and j in (5, 10):
                # Q block qb+1 woven into this qb (one co half per slot)
                co = 0 if j == 5 else 1
                nbw = qb + 1
                pqw = ps_o.tile([P, QB], F32, name="pqw", tag="po")
                nc.tensor.matmul(
                    pqw, lhsT=w8["wqt"][:, :, co * P:(co + 1) * P],
                    rhs=x8_sb[:, :, nbw * QB:(nbw + 1) * QB],
                    start=True, stop=True, perf_mode=DR,
                )
                nc.vector.tensor_scalar(
                    out=q_sb[:, co, nbw * QB:(nbw + 1) * QB], in0=pqw,
                    scalar1=0.125, scalar2=be_q[co], op0=Alu.mult, op1=Alu.add,
                )
            if j < NPAIR:
                sc = ps_sc.tile([P, 2, QB], F32, name="sc", tag="sc")
                for h in range(2):
                    kt = 2 * j + h
                    nc.tensor.matmul(
                        sc[:, h, :],
                        lhsT=k_sb[:, :, kt * P:(kt + 1) * P],
                        rhs=q_sb[:, :, qb * QB:(qb + 1) * QB],
                        start=True, stop=True, perf_mode=DR,
                    )
                nc.scalar.activation(
                    p_sb[:, 2 * j:2 * j + 2, :], sc, Act.Exp,
                    bias=shift_col, scale=SCALE,
                )
            if j >= 1:
                pj = j - 1
                p_pair = p_sb[:, 2 * pj:2 * pj + 2, :]
                nc.tensor.matmul(
                    dps, lhsT=ones8, rhs=p_pair,
                    start=(pj == 0), stop=(pj == NPAIR - 1),
                    perf_mode=DR, skip_group_check=True,
                )
                for ch in range(2):
                    nc.tensor.matmul(
                        aps[ch],
                        lhsT=v_sb[:, 2 * pj:2 * pj + 2, ch * P:(ch + 1) * P],
                        rhs=p_pair,
                        start=(pj == 0), stop=(pj == NPAIR - 1),
                        perf_mode=DR, skip_group_check=True,
                    )
        pending = (qb, dps, aps)
    epilogue(*pending, final=True)

    for pool in (ps_o, ps_d, ps_acc, ps_sc, work, pblk, small, stage, const):
        pool.release()


def build_program():
    global _NC
    if _NC is not None:
        return _NC
    nc = bacc.Bacc("TRN2", target_bir_lowering=False, debug=False,
                   num_devices=NCORES)
    d = {
        "x": nc.dram_tensor("x", [C, N], BF16, kind="ExternalInput"),
        "x8": nc.dram_tensor("x8", [C, N], FP8, kind="ExternalInput"),
        "wqt": nc.dram_tensor("wqt", [C, C], F32, kind="ExternalInput"),
        "wkt": nc.dram_tensor("wkt", [C, C], F32, kind="ExternalInput"),
        "wvt": nc.dram_tensor("wvt", [C, C], F32, kind="ExternalInput"),
        "wot": nc.dram_tensor("wot", [C, C], F32, kind="ExternalInput"),
        "bq": nc.dram_tensor("bq", [C, 1], F32, kind="ExternalInput"),
        "bv": nc.dram_tensor("bv", [C, 1], F32, kind="ExternalInput"),
        "bo": nc.dram_tensor("bo", [C, 1], F32, kind="ExternalInput"),
        "gind": nc.dram_tensor("gind", [P, 2 * G], F32, kind="ExternalInput"),
        "gindT": nc.dram_tensor("gindT", [G, 2 * P], F32,
                                kind="ExternalInput"),
        "gb_cols": nc.dram_tensor("gb_cols", [P, 4], F32,
                                  kind="ExternalInput"),
        "out": nc.dram_tensor("out", [C, NQ], F32, kind="ExternalOutput"),
    }
    with tile.TileContext(nc) as tc:
        _body(tc, d)
    nc.compile()
    _NC = nc
    return nc


def make_in_maps(x, gamma, beta, wq, bq, wk, bk, wv, bv, wo, bo):
    f32c = lambda a: np.ascontiguousarray(np.asarray(a, dtype=np.float32))
    x = f32c(x)
    # group indicator matrices: channel p of input half h belongs to group
    # p//CPG + G//2*h; gind contracts channels -> groups, gindT broadcasts
    # groups -> channels
    gind = np.zeros((P, 2 * G), np.float32)
    gindT = np.zeros((G, 2 * P), np.float32)
    for h in range(2):
        for p in range(P):
            g = p // CPG + (G // 2) * h
            gind[p, G * h + g] = 1.0
            gindT[g, P * h + p] = 1.0
    gam2 = f32c(gamma).reshape(2, P)
    bet2 = f32c(beta).reshape(2, P)
    gb_cols = np.stack([gam2[0], gam2[1], bet2[0], bet2[1]], axis=1)
    base = {
        "wqt": f32c(np.asarray(wq, np.float32).T),
        "wkt": f32c(np.asarray(wk, np.float32).T),
        "wvt": f32c(np.asarray(wv, np.float32).T),
        "wot": f32c(np.asarray(wo, np.float32).T),
        "bq": f32c(bq).reshape(C, 1),
        "bv": f32c(bv).reshape(C, 1),
        "bo": f32c(bo).reshape(C, 1),
        "gind": gind,
        "gindT": gindT,
        "gb_cols": np.ascontiguousarray(gb_cols),
    }
    import ml_dtypes

    in_maps = []
    for core in range(NCORES):
        b, h = divmod(core, 2)
        xb = x[b].reshape(C, N)
        if h:
            xb = np.concatenate([xb[:, NQ:], xb[:, :NQ]], axis=1)
        in_maps.append({
            **base,
            "x": np.ascontiguousarray(xb.astype(ml_dtypes.bfloat16)),
            "x8": np.ascontiguousarray(xb.astype(ml_dtypes.float8_e4m3)),
        })
    return in_maps


def kernel(x, gamma, beta, wq, bq, wk, bk, wv, bv, wo, bo):
    global LAST_RESULTS
    from concourse.bass_utils import run_bass_kernel_spmd

    nc = build_program()
    in_maps = make_in_maps(x, gamma, beta, wq, bq, wk, bk, wv, bv, wo, bo)
    res = run_bass_kernel_spmd(nc, in_maps, core_ids=list(range(NCORES)))
    LAST_RESULTS = res
    out = np.empty((B, C, N), np.float32)
    for core in range(NCORES):
        b, h = divmod(core, 2)
        out[b][:, h * NQ:(h + 1) * NQ] = res.results[core]["out"]
    return out.reshape(B, C, H, W)
